# revision 5
# baseline (speedup 1.0000x reference)
"""Trainium2 Bass kernel for ChunkedGeoSparseLinear (gather-mode sparse linear).

out[n, o] = sum_k x[n, idx[o, k]] * w[o, k] + b[o]
  x: (4096, 4096) f32, idx: (4096, 16) i64, w: (4096, 16) f32, b: (4096,) f32

Strategy (output-parallel, 8 cores): core d owns outputs [512d, 512d+512)
for ALL 4096 batch rows.

  - Host: xT = x.T cast to fp8 e3m4 (full 16 MB table, every core gets it).
    Per core, per 128-output group: dedup the 2048 tap rows (~1612 distinct),
    pad to bpg*128, and prebuild dense bf16 lhsT blocks
    W[p, o] = sum of w[o, m] over taps with idx[o, m] == row_p.
  - Device: dma_gather (SWDGE) pulls the ~1664 distinct 4 KB fp8 rows per
    group from HBM into SBUF ([128, bpg, 4096] tiles). Big rows keep the
    Q7 descriptor-generation cost negligible (~13 calls x ~1 us).
  - PE: per group, bpg matmuls per 512-col n-chunk with the dense bf16
    lhsT blocks accumulate into PSUM [128 outputs, 512 n]. bf16 x fp8e3
    products are exact in fp32 PSUM, so the only quantization error is
    the e3m4 rounding of x (~1.6e-2 rel).
  - ScalarE drains PSUM with the bias add into bf16; DMA writes the
    [512, 4096] outT slab; host transposes, upcasts, and stacks.
"""

import sys

import numpy as np
import ml_dtypes

for _p in ("/opt/trn_rl_repo", "/opt/pypackages"):
    if _p not in sys.path:
        sys.path.append(_p)

N = 4096
IN_F = 4096
OUT_F = 4096
K = 16
NCORES = 8
OUT_PC = OUT_F // NCORES      # 512 outputs per core
GPC = OUT_PC // 128           # 4 psum groups of 128 outputs per core
NCHUNK = 512                  # matmul rhs / psum columns
NHALF = 2048                  # n-columns per (group, half) drain unit
TPC = 512                     # taps per dma_gather call (SWDGE ring limit)

_CACHE = {}


def _build(reps: int = 1, bpg: int = 13):
    """Build + compile the per-core Bass program (SPMD: same program, 8 cores).

    bpg: gather/lhsT blocks of 128 rows per 128-output group (covers the
    group's distinct tap rows, padded with row 0 / zero weights).
    """
    import concourse.bacc as bacc
    import concourse.mybir as mybir
    import concourse.tile as tile

    dt = mybir.dt
    nc = bacc.Bacc("TRN2", debug=False, num_devices=NCORES,
                   enable_partition_id=False, num_swdge_queues=4)

    rows_pg = bpg * 128                       # padded rows per group
    nidx = GPC * rows_pg                      # gathered rows per core

    xt = nc.dram_tensor("xt", [IN_F, N], dt.float8e3, kind="ExternalInput")
    idxs = nc.dram_tensor("idxs", [128, nidx // 16], dt.int16,
                          kind="ExternalInput")
    wblk = nc.dram_tensor("wblk", [128, GPC * bpg, 128], dt.bfloat16,
                          kind="ExternalInput")
    bias = nc.dram_tensor("bias", [128, GPC], dt.float32, kind="ExternalInput")
    outT = nc.dram_tensor("outT", [OUT_PC, N], dt.bfloat16,
                          kind="ExternalOutput")
    # reps-dependent output shape keeps timing variants from aliasing in the
    # executable cache (the cache key ignores the embedded BIR)
    nc.dram_tensor("repstag", [1, reps], dt.float32, kind="ExternalOutput")

    with tile.TileContext(nc) as tc:
        with (
            tc.tile_pool(name="singles", bufs=1) as singles,
            tc.tile_pool(name="gpool", bufs=2) as gpool,
            tc.tile_pool(name="ppool", bufs=2, space="PSUM") as ppool,
            tc.tile_pool(name="opool", bufs=4) as opool,
        ):
            idxs_sb = singles.tile([128, nidx // 16], dt.int16)
            nc.sync.dma_start(idxs_sb[:], idxs[:])
            wblk_sb = singles.tile([128, GPC * bpg, 128], dt.bfloat16)
            nc.sync.dma_start(wblk_sb[:], wblk[:])
            bias_sb = singles.tile([128, GPC], dt.float32)
            nc.sync.dma_start(bias_sb[:], bias[:])

            def body(_i=None):
                qn = [0]
                for g in range(GPC):
                    gt = gpool.tile([128, bpg, N], dt.float8e3)
                    done = 0
                    while done < rows_pg:
                        n_i = min(TPC, rows_pg - done)
                        off = g * rows_pg + done
                        nc.gpsimd.dma_gather(
                            gt[:, done // 128:(done + n_i) // 128, :],
                            xt[:],
                            idxs_sb[:, off // 16:(off + n_i) // 16],
                            n_i, n_i, N,
                            queue_num=qn[0] % 4,
                        )
                        qn[0] += 1
                        done += n_i
                    for h in range(2):
                        ps = [ppool.tile([128, NCHUNK], dt.float32,
                                         name=f"p{c}")
                              for c in range(NHALF // NCHUNK)]
                        for b in range(bpg):
                            for c, p in enumerate(ps):
                                col = h * NHALF + c * NCHUNK
                                nc.tensor.matmul(
                                    p[:], wblk_sb[:, g * bpg + b, :],
                                    gt[:, b, col:col + NCHUNK],
                                    start=(b == 0), stop=(b == bpg - 1))
                        o = opool.tile([128, NHALF], dt.bfloat16)
                        for c, p in enumerate(ps):
                            nc.scalar.activation(
                                o[:, c * NCHUNK:(c + 1) * NCHUNK], p[:],
                                mybir.ActivationFunctionType.Identity,
                                bias=bias_sb[:, g:g + 1])
                        nc.sync.dma_start(
                            outT[g * 128:(g + 1) * 128,
                                 h * NHALF:(h + 1) * NHALF], o[:])

            if reps == 1:
                body()
            else:
                with tc.For_i(0, reps, 1):
                    body()

    nc.compile()
    return nc


def _prep_inputs(x, in_index_per_out, weight, bias):
    """Host-side data prep: fp8 table + per-core dedup indices/weights."""
    idx = np.asarray(in_index_per_out).astype(np.int64)
    w = np.asarray(weight).astype(np.float32)
    b = np.asarray(bias).astype(np.float32)

    xT8 = np.ascontiguousarray(np.asarray(x).astype(np.float32).T
                               .astype(ml_dtypes.float8_e3m4))  # (IN_F, N)

    # blocks per group: max distinct tap rows over all (core, group)
    ngrp = OUT_F // 128
    uniq = [np.unique(idx[gg * 128:(gg + 1) * 128]) for gg in range(ngrp)]
    bpg = int(np.ceil(max(len(u) for u in uniq) / 128))
    rows_pg = bpg * 128

    idxs_l, wblk_l, bias_l = [], [], []
    cols128 = np.broadcast_to(np.arange(128)[:, None], (128, K))
    for d in range(NCORES):
        flat_rows = np.zeros(GPC * rows_pg, dtype=np.int64)
        wb = np.zeros((GPC * bpg, 128, 128), dtype=np.float32)  # [blk, p, o]
        for g in range(GPC):
            gg = d * GPC + g
            rows_u = uniq[gg]
            taps = idx[gg * 128:(gg + 1) * 128]              # (128, K)
            pos = np.searchsorted(rows_u, taps)              # (128, K)
            wg = np.zeros((rows_pg, 128), dtype=np.float32)
            np.add.at(wg, (pos.ravel(), cols128.ravel()),
                      w[gg * 128:(gg + 1) * 128].ravel())
            flat_rows[g * rows_pg:g * rows_pg + len(rows_u)] = rows_u
            wb[g * bpg:(g + 1) * bpg] = wg.reshape(bpg, 128, 128)
        wrap = flat_rows.reshape(-1, 16).T                   # [16, nidx//16]
        idxs_l.append(np.tile(wrap, (8, 1)).astype(np.int16))
        wblk_l.append(np.ascontiguousarray(
            wb.transpose(1, 0, 2)).astype(ml_dtypes.bfloat16))
        bias_l.append(np.ascontiguousarray(
            b[d * OUT_PC:(d + 1) * OUT_PC].reshape(GPC, 128).T))
    return xT8, idxs_l, wblk_l, bias_l, bpg


def _in_maps(prep):
    xT8, idxs_l, wblk_l, bias_l, bpg = prep
    return [
        {"xt": xT8, "idxs": idxs_l[d], "wblk": wblk_l[d], "bias": bias_l[d]}
        for d in range(NCORES)
    ]


def kernel(x, in_index_per_out, weight, bias):
    from concourse import bass_utils

    prep = _prep_inputs(x, in_index_per_out, weight, bias)
    bpg = prep[-1]

    key = ("nc", 1, bpg)
    if key not in _CACHE:
        _CACHE[key] = _build(reps=1, bpg=bpg)
    nc = _CACHE[key]

    res = bass_utils.run_bass_kernel_spmd(nc, _in_maps(prep),
                                          core_ids=list(range(NCORES)))
    out = np.empty((N, OUT_F), dtype=np.float32)
    for d in range(NCORES):
        out[:, d * OUT_PC:(d + 1) * OUT_PC] = (
            res.results[d]["outT"].astype(np.float32).T)
    return out


# revision 8
# speedup vs baseline: 1.2539x; 1.2539x over previous
"""Trainium2 Bass kernel for ChunkedGeoSparseLinear (gather-mode sparse linear).

out[n, o] = sum_k x[n, idx[o, k]] * w[o, k] + b[o]
  x: (4096, 4096) f32, idx: (4096, 16) i64, w: (4096, 16) f32, b: (4096,) f32

Strategy (output-parallel, 8 cores): core d owns outputs [512d, 512d+512)
for ALL 4096 batch rows, as 2 pairs of 128-output groups.

  - Host: xT = x.T cast to fp8 e3m4 (full 16 MB table, every core gets it).
    Per pair of groups (A, B): partition the union of their tap rows into
    [shared | A-only | B-only] segments (shared rows gathered ONCE, fed to
    both groups' matmuls), pad each segment to NSH/NEX 128-row blocks, and
    prebuild dense bf16 lhsT blocks W[p, o] = sum of w[o, m] over taps with
    idx[o, m] == row_p.  Shared rows that overflow NSH blocks spill into
    both exclusive lists.  This cuts gather DMA ~19% at identical PE cost.
  - Device: dma_gather (SWDGE) pulls the 4 KB fp8 rows from HBM into SBUF
    ([128, NBLK, 4096] tiles; 512-idx calls keep the SWDGE ring happy).
  - PE (chunk-major; PSUM-bank switches only every 13 matmuls to avoid
    HAM bank-cycling): per group and 512-col chunk, 13 accumulating
    matmuls with dense bf16 lhsT against the fp8 rhs.  bf16 x fp8e3
    products are exact in fp32 PSUM, so the only quantization error is
    the e3m4 rounding of x (~1.6e-2 rel).
  - ScalarE drains PSUM with the bias add into bf16; DMA writes the
    [512, 4096] outT slab; host transposes, upcasts, and stacks.
"""

import sys

import numpy as np
import ml_dtypes

for _p in ("/opt/trn_rl_repo", "/opt/pypackages"):
    if _p not in sys.path:
        sys.path.append(_p)

N = 4096
IN_F = 4096
OUT_F = 4096
K = 16
NCORES = 8
OUT_PC = OUT_F // NCORES      # 512 outputs per core
GPC = OUT_PC // 128           # 4 groups of 128 outputs per core
NPAIR = GPC // 2              # 2 group-pairs per core
NCHUNK = 512                  # matmul rhs / psum columns
TPC = 512                     # taps per dma_gather call (SWDGE ring limit)

_CACHE = {}


def _build(reps: int, nsh: int, nex: int):
    """Build + compile the per-core Bass program (SPMD: same program, 8 cores).

    nsh/nex: 128-row blocks per pair for the shared / each exclusive segment.
    """
    import concourse.bacc as bacc
    import concourse.mybir as mybir
    import concourse.tile as tile

    dt = mybir.dt
    nc = bacc.Bacc("TRN2", debug=False, num_devices=NCORES,
                   enable_partition_id=False, num_swdge_queues=4)

    nblk = nsh + 2 * nex                      # gather blocks per pair
    gb = nsh + nex                            # lhsT blocks per group
    rows_pp = nblk * 128                      # gathered rows per pair
    nidx = NPAIR * rows_pp                    # gathered rows per core
    nchk = N // NCHUNK

    xt = nc.dram_tensor("xt", [IN_F, N], dt.float8e3, kind="ExternalInput")
    idxs = nc.dram_tensor("idxs", [128, nidx // 16], dt.int16,
                          kind="ExternalInput")
    wblk = nc.dram_tensor("wblk", [128, 2 * NPAIR * gb, 128], dt.bfloat16,
                          kind="ExternalInput")
    bias = nc.dram_tensor("bias", [128, GPC], dt.float32, kind="ExternalInput")
    outT = nc.dram_tensor("outT", [OUT_PC, N], dt.bfloat16,
                          kind="ExternalOutput")
    # reps-dependent output shape keeps timing variants from aliasing in the
    # executable cache (the cache key ignores the embedded BIR)
    nc.dram_tensor("repstag", [1, reps], dt.float32, kind="ExternalOutput")

    with tile.TileContext(nc) as tc:
        with (
            tc.tile_pool(name="singles", bufs=1) as singles,
            tc.tile_pool(name="gpool", bufs=2) as gpool,
            tc.tile_pool(name="ppool", bufs=2, space="PSUM") as ppool,
            tc.tile_pool(name="opool", bufs=1) as opool,
        ):
            idxs_sb = singles.tile([128, nidx // 16], dt.int16)
            nc.sync.dma_start(idxs_sb[:], idxs[:])
            wblk_sb = singles.tile([128, 2 * NPAIR * gb, 128], dt.bfloat16)
            nc.sync.dma_start(wblk_sb[:], wblk[:])
            bias_sb = singles.tile([128, GPC], dt.float32)
            nc.sync.dma_start(bias_sb[:], bias[:])

            def body(_i=None):
                qn = [0]
                for pair in range(NPAIR):
                    gt = gpool.tile([128, nblk, N], dt.float8e3)
                    done = 0
                    while done < rows_pp:
                        n_i = min(TPC, rows_pp - done)
                        off = pair * rows_pp + done
                        nc.gpsimd.dma_gather(
                            gt[:, done // 128:(done + n_i) // 128, :],
                            xt[:],
                            idxs_sb[:, off // 16:(off + n_i) // 16],
                            n_i, n_i, N,
                            queue_num=qn[0] % 4,
                        )
                        qn[0] += 1
                        done += n_i
                    # side 0 (A): gather blocks [0, gb); lhsT cols base
                    # side 1 (B): gather blocks [0, nsh) + [gb, nblk)
                    sides = [
                        (0, list(range(gb))),
                        (1, list(range(nsh)) + list(range(gb, nblk))),
                    ]
                    os_ = [opool.tile([128, N], dt.bfloat16, name=f"o{s}")
                           for s in range(2)]
                    for c in range(nchk):
                        for side, gblocks in sides:
                            g = pair * 2 + side
                            p = ppool.tile([128, NCHUNK], dt.float32,
                                           name=f"p{side}{c % 2}")
                            wbase = (pair * 2 + side) * gb
                            for j, bg in enumerate(gblocks):
                                nc.tensor.matmul(
                                    p[:], wblk_sb[:, wbase + j, :],
                                    gt[:, bg, c * NCHUNK:(c + 1) * NCHUNK],
                                    start=(j == 0), stop=(j == gb - 1))
                            nc.scalar.activation(
                                os_[side][:, c * NCHUNK:(c + 1) * NCHUNK],
                                p[:],
                                mybir.ActivationFunctionType.Identity,
                                bias=bias_sb[:, g:g + 1])
                    for side in range(2):
                        g = pair * 2 + side
                        nc.sync.dma_start(outT[g * 128:(g + 1) * 128, :],
                                          os_[side][:])

            if reps == 1:
                body()
            else:
                with tc.For_i(0, reps, 1):
                    body()

    nc.compile()
    return nc


def _pick_blocks(pairs):
    """Choose (nsh, nex): minimize matmuls (nsh+nex), then rows (nsh+2nex)."""
    best = None
    max_sh = max(len(sh) for sh, _, _ in pairs)
    for nsh in range(0, (max_sh + 127) // 128 + 1):
        nex = 0
        for sh, ea, eb in pairs:
            ov = max(0, len(sh) - nsh * 128)
            nex = max(nex, -(-(len(ea) + ov) // 128), -(-(len(eb) + ov) // 128))
        cost = (nsh + nex, nsh + 2 * nex)
        if best is None or cost < best[0]:
            best = (cost, nsh, nex)
    return best[1], best[2]


def _prep_inputs(x, in_index_per_out, weight, bias):
    """Host-side data prep: fp8 table + per-core pair-shared dedup structure."""
    idx = np.asarray(in_index_per_out).astype(np.int64)
    w = np.asarray(weight).astype(np.float32)
    b = np.asarray(bias).astype(np.float32)

    xT8 = np.ascontiguousarray(np.asarray(x).astype(np.float32).T
                               .astype(ml_dtypes.float8_e3m4))  # (IN_F, N)

    # per (core, pair): shared / exclusive row sets
    pair_sets = []
    for d in range(NCORES):
        for pr in range(NPAIR):
            gA = d * GPC + pr * 2
            sA = np.unique(idx[gA * 128:(gA + 1) * 128])
            sB = np.unique(idx[(gA + 1) * 128:(gA + 2) * 128])
            sh = np.intersect1d(sA, sB)
            pair_sets.append((sh, np.setdiff1d(sA, sh), np.setdiff1d(sB, sh)))
    nsh, nex = _pick_blocks(pair_sets)
    nblk = nsh + 2 * nex
    gb = nsh + nex

    def pad(a, ln):
        out = np.zeros(ln, dtype=np.int64)
        out[:len(a)] = a
        return out

    cols128 = np.broadcast_to(np.arange(128)[:, None], (128, K))
    idxs_l, wblk_l, bias_l = [], [], []
    for d in range(NCORES):
        flat_rows = np.zeros(NPAIR * nblk * 128, dtype=np.int64)
        wb = np.zeros((2 * NPAIR * gb, 128, 128), dtype=np.float32)
        for pr in range(NPAIR):
            sh, ea, eb = pair_sets[d * NPAIR + pr]
            kept = sh[:nsh * 128]
            spill = sh[nsh * 128:]
            eaL = np.sort(np.concatenate([ea, spill]))
            ebL = np.sort(np.concatenate([eb, spill]))
            rows = np.concatenate([pad(kept, nsh * 128), pad(eaL, nex * 128),
                                   pad(ebL, nex * 128)])
            flat_rows[pr * nblk * 128:(pr + 1) * nblk * 128] = rows
            for side, exL in ((0, eaL), (1, ebL)):
                gg = d * GPC + pr * 2 + side
                taps = idx[gg * 128:(gg + 1) * 128]          # (128, K)
                pk = np.searchsorted(kept, taps)
                in_kept = (pk < len(kept)) & (
                    kept[np.minimum(pk, max(len(kept) - 1, 0))] == taps
                ) if len(kept) else np.zeros_like(taps, bool)
                pos = np.where(in_kept, pk,
                               nsh * 128 + np.searchsorted(exL, taps))
                wg = np.zeros((gb * 128, 128), dtype=np.float32)
                np.add.at(wg, (pos.ravel(), cols128.ravel()),
                          w[gg * 128:(gg + 1) * 128].ravel())
                wb[(pr * 2 + side) * gb:(pr * 2 + side + 1) * gb] = (
                    wg.reshape(gb, 128, 128))
        wrap = flat_rows.reshape(-1, 16).T                   # [16, nidx//16]
        idxs_l.append(np.tile(wrap, (8, 1)).astype(np.int16))
        wblk_l.append(np.ascontiguousarray(
            wb.transpose(1, 0, 2)).astype(ml_dtypes.bfloat16))
        bias_l.append(np.ascontiguousarray(
            b[d * OUT_PC:(d + 1) * OUT_PC].reshape(GPC, 128).T))
    return xT8, idxs_l, wblk_l, bias_l, (nsh, nex)


def _in_maps(prep):
    xT8, idxs_l, wblk_l, bias_l, _ = prep
    return [
        {"xt": xT8, "idxs": idxs_l[d], "wblk": wblk_l[d], "bias": bias_l[d]}
        for d in range(NCORES)
    ]


def kernel(x, in_index_per_out, weight, bias):
    from concourse import bass_utils

    prep = _prep_inputs(x, in_index_per_out, weight, bias)
    nsh, nex = prep[-1]

    key = ("nc", 1, nsh, nex)
    if key not in _CACHE:
        _CACHE[key] = _build(1, nsh, nex)
    nc = _CACHE[key]

    res = bass_utils.run_bass_kernel_spmd(nc, _in_maps(prep),
                                          core_ids=list(range(NCORES)))
    out = np.empty((N, OUT_F), dtype=np.float32)
    for d in range(NCORES):
        out[:, d * OUT_PC:(d + 1) * OUT_PC] = (
            res.results[d]["outT"].astype(np.float32).T)
    return out
